# revision 1
# baseline (speedup 1.0000x reference)
"""Degraded bicycle rollout kernel for Trainium2 (8 NeuronCores, data-parallel on batch).

Math restructure (validated vs reference in numpy):
  - All control-dependent transcendentals (tanh/sigmoid/tan/arctan) hoisted out
    of the sequential scan and computed in parallel over (b,l,h).
  - The scan recurrence only propagates through `speed`, because
    vx^2+vy^2 == speed2^2 exactly. The exact per-step map
        s2_t = max(s_t + accDT_t, 0);  s_{t+1} = sqrt(s2_t^2 + 1e-6)
    runs as an 80-step serial chain (DVE add, DVE relu*u, ACT sqrt).
  - psi / px / py are per-rollout prefix sums -> hardware tensor_tensor_scan
    (segmented via a multiply-mask reset).
  - ax/ay are shifted differences; channels 8..11 and t=0 are control-only.

Layout per core: 4096 rollouts = 128 partitions x 32; partition p holds
rollouts p*32..p*32+31, all from batch bc = p//2 (so per-batch scales are
per-partition scalars). Free dim is rollout-major: f = n*80 + t.
Output staged channel-interleaved in SBUF, DMA'd in 8 rollout-chunks.
"""

import sys

sys.path.insert(0, "/opt/trn_rl_repo")

import numpy as np

B, L, H = 512, 64, 80
NCORES = 8
BC = B // NCORES          # 64 batches per core
R = BC * L                # 4096 rollouts per core
P = 128
NPT = R // P              # 32 rollouts per partition
F = NPT * H               # 2560 elements per partition (dense tiles)
HP1 = H + 1
CW = 12                   # output channels
SW = HP1 * CW             # 972 staging words per rollout
G = 8                     # output chunks (rollout groups)
NG = NPT // G             # 4 rollouts per partition per chunk
CF = NG * H               # 320 free elems per chunk slice
CHW = NG * SW             # staging words per partition per chunk
DT = 0.1
WB = 2.8
PI = float(np.pi)

_BUILT = None


def _build_kernel():
    import concourse.bass as bass
    import concourse.bacc as bacc
    import concourse.mybir as mybir
    from concourse.tile import TileContext
    from concourse.tile_rust import add_dep_helper

    f32 = mybir.dt.float32
    A = mybir.AluOpType
    AF = mybir.ActivationFunctionType

    nc = bacc.Bacc(None, target_bir_lowering=False)
    ctrl_d = nc.declare_dram_parameter("ctrl", [P, NPT * H * 3], f32, isOutput=False)
    x0_d = nc.declare_dram_parameter("x0p", [P, 12], f32, isOutput=False)
    deg_d = nc.declare_dram_parameter("degp", [P, 5], f32, isOutput=False)
    out_d = nc.declare_dram_parameter("out", [P, NPT * SW], f32, isOutput=True)

    with TileContext(nc) as tc:
        v = nc.vector
        sc = nc.scalar
        gp = nc.gpsimd
        sy = nc.sync

        with tc.tile_pool(name="pers", bufs=1) as pp, \
             tc.tile_pool(name="work", bufs=3) as wp, \
             tc.tile_pool(name="late", bufs=1) as lp, \
             tc.tile_pool(name="stgp", bufs=2) as sp, \
             tc.tile_pool(name="cs", bufs=2) as cp, \
             tc.tile_pool(name="psp", bufs=6, space="PSUM") as ps:

            # ---------- small constants ----------
            x0s = pp.tile([P, 12], f32, tag="x0s")
            degs = pp.tile([P, 5], f32, tag="degs")
            sy.dma_start(out=x0s[:], in_=x0_d[:])
            sy.dma_start(out=degs[:], in_=deg_d[:])

            vecs = pp.tile([P, 16], f32, tag="vecs")
            steer = vecs[:, 0:1]
            brake = vecs[:, 1:2]
            thr = vecs[:, 2:3]
            b65 = vecs[:, 3:4]
            t28 = vecs[:, 4:5]
            lo75 = vecs[:, 5:6]
            f981 = vecs[:, 6:7]
            vx010 = vecs[:, 7:8]
            vy010 = vecs[:, 8:9]
            s0q = vecs[:, 9:10]
            s0v = vecs[:, 10:11]
            tmpa = vecs[:, 11:12]
            tmpb = vecs[:, 12:13]
            epsv = vecs[:, 13:14]
            hpiv = vecs[:, 14:15]
            psi0 = x0s[:, 2:3]
            px0 = x0s[:, 0:1]
            py0 = x0s[:, 1:2]
            vx0 = x0s[:, 3:4]
            vy0 = x0s[:, 4:5]

            v.tensor_scalar(steer, degs[:, 0:1], 0.05, None, A.max)
            v.tensor_scalar(brake, degs[:, 1:2], 0.05, None, A.max)
            v.tensor_scalar(thr, degs[:, 2:3], 0.05, None, A.max)
            v.tensor_scalar(b65, degs[:, 1:2], 0.05, 0.65, A.max, A.mult)
            v.tensor_scalar(t28, degs[:, 2:3], 0.05, 0.28, A.max, A.mult)
            v.tensor_scalar(lo75, degs[:, 4:5], 0.1, -0.75, A.max, A.mult)
            v.tensor_scalar(f981, degs[:, 4:5], 0.1, 9.81, A.max, A.mult)
            v.tensor_scalar(vx010, vx0, 10.0, None, A.mult)
            v.tensor_scalar(vy010, vy0, 10.0, None, A.mult)
            v.tensor_tensor(tmpa, vx0, vx0, A.mult)
            v.tensor_tensor(tmpb, vy0, vy0, A.mult)
            v.tensor_tensor(s0q, tmpa, tmpb, A.add)
            v.memset(epsv, 1e-6)
            v.memset(hpiv, PI / 2)

            ones32 = pp.tile([P, NPT], f32, tag="ones32")
            v.memset(ones32[:], 1.0)

            # per-chunk scan mask: 0 at t==0 of each rollout, 1 elsewhere
            maskc = pp.tile([P, CF], f32, tag="maskc")
            v.memset(maskc[:], 1.0)
            mc3 = maskc[:].rearrange("p (n h) -> p n h", n=NG)
            v.memset(mc3[:, :, 0:1], 0.0)

            # ---------- load controls ----------
            ctrl = wp.tile([P, NPT * H * 3], f32, tag="big", bufs=1)
            CH2 = (NPT // 2) * H * 3
            sy.dma_start(out=ctrl[:, 0:CH2], in_=ctrl_d[:, 0:CH2])
            sy.dma_start(out=ctrl[:, CH2:], in_=ctrl_d[:, CH2:])
            c4 = ctrl[:].rearrange("p (n h c) -> p n h c", n=NPT, h=H)
            u0 = c4[:, :, :, 0]
            u1 = c4[:, :, :, 1]
            u2 = c4[:, :, :, 2]

            # ---------- phase A: control transforms ----------
            HF = F // 2
            HN = NPT // 2
            sg1 = wp.tile([P, F], f32, tag="W2", bufs=1)
            sg13 = sg1[:].rearrange("p (n h) -> p n h", n=NPT)
            i_sg1a = sc.activation(sg13[:, 0:HN, :], u1[:, 0:HN, :], AF.Sigmoid)
            sg2 = wp.tile([P, F], f32, tag="W3", bufs=1)
            sg23 = sg2[:].rearrange("p (n h) -> p n h", n=NPT)
            i_sg2a = sc.activation(sg23[:, 0:HN, :], u2[:, 0:HN, :], AF.Sigmoid)
            th = wp.tile([P, F], f32, tag="W1", bufs=1)
            th3 = th[:].rearrange("p (n h) -> p n h", n=NPT)
            i_tanh = sc.activation(th3, u0, AF.Tanh)
            i_sg1b = sc.activation(sg13[:, HN:, :], u1[:, HN:, :], AF.Sigmoid)
            i_sg2b = sc.activation(sg23[:, HN:, :], u2[:, HN:, :], AF.Sigmoid)

            fb65 = wp.tile([P, F], f32, tag="W1p", bufs=1)
            v.tensor_scalar(fb65[:, 0:HF], sg1[:, 0:HF], b65, None, A.mult)
            t3 = wp.tile([P, F], f32, tag="W2p", bufs=1)
            v.scalar_tensor_tensor(t3[:, 0:HF], sg2[:, 0:HF], t28,
                                   fb65[:, 0:HF], A.mult, A.subtract)
            accDT = wp.tile([P, F], f32, tag="accdt", bufs=1)
            v.tensor_scalar(accDT[:, 0:HF], t3[:, 0:HF], 0.3, lo75,
                            A.min, A.max)
            v.tensor_scalar(fb65[:, HF:], sg1[:, HF:], b65, None, A.mult)
            v.scalar_tensor_tensor(t3[:, HF:], sg2[:, HF:], t28,
                                   fb65[:, HF:], A.mult, A.subtract)
            v.tensor_scalar(accDT[:, HF:], t3[:, HF:], 0.3, lo75,
                            A.min, A.max)

            delta = lp.tile([P, F], f32, tag="delta")
            v.tensor_scalar(delta[:], th[:], steer, None, A.mult)
            dc = wp.tile([P, F], f32, tag="W4", bufs=1)
            v.tensor_scalar(dc[:], delta[:], 0.75, -0.75, A.min, A.max)
            fb = lp.tile([P, F], f32, tag="fb")
            v.tensor_scalar(fb[:], sg1[:], brake, None, A.mult)
            fx = lp.tile([P, F], f32, tag="fx")
            v.tensor_scalar(fx[:], sg2[:], thr, None, A.mult)

            # ---------- phase B: serial speed recurrence (exact) ----------
            i_s0 = sc.activation(s0v, s0q, AF.Sqrt, bias=epsv)
            add_dep_helper(i_s0.ins, i_tanh.ins, reason="act set sig->sqrt")
            add_dep_helper(i_s0.ins, i_sg1b.ins, reason="act set sig->sqrt")
            add_dep_helper(i_s0.ins, i_sg2b.ins, reason="act set sig->sqrt")

            HNP = NPT // 2
            s_curA = pp.tile([P, HNP], f32, tag="s_a0")
            s_nxtA = pp.tile([P, HNP], f32, tag="s_a1")
            u_tA = pp.tile([P, HNP], f32, tag="u_a")
            s_curB = pp.tile([P, HNP], f32, tag="s_b0")
            s_nxtB = pp.tile([P, HNP], f32, tag="s_b1")
            u_tB = pp.tile([P, HNP], f32, tag="u_b")
            v.tensor_scalar(s_curA[:], ones32[:, 0:HNP], s0v, None, A.mult)
            v.tensor_scalar(s_curB[:], ones32[:, 0:HNP], s0v, None, A.mult)

            qA = pp.tile([P, HNP], f32, tag="q_a")
            qB = pp.tile([P, HNP], f32, tag="q_b")
            ufull = wp.tile([P, F], f32, tag="big", bufs=1)
            a3 = accDT[:].rearrange("p (n h) -> p n h", n=NPT)
            u3 = ufull[:].rearrange("p (n h) -> p n h", n=NPT)
            last_sqrt = None
            for t in range(H):
                v.tensor_tensor(u3[:, 0:HNP, t], s_curA[:], a3[:, 0:HNP, t],
                                A.add)
                v.scalar_tensor_tensor(qA[:], u3[:, 0:HNP, t], 0.0,
                                       u3[:, 0:HNP, t], A.max, A.mult)
                i_sqA = sc.activation(s_nxtA[:], qA[:], AF.Sqrt, bias=epsv)
                v.tensor_tensor(u3[:, HNP:, t], s_curB[:], a3[:, HNP:, t],
                                A.add)
                v.scalar_tensor_tensor(qB[:], u3[:, HNP:, t], 0.0,
                                       u3[:, HNP:, t], A.max, A.mult)
                i_sqB = sc.activation(s_nxtB[:], qB[:], AF.Sqrt, bias=epsv)
                if last_sqrt is None:
                    add_dep_helper(i_sqA.ins, i_s0.ins, reason="chain after s0")
                    add_dep_helper(i_sqB.ins, i_s0.ins, reason="chain after s0")
                s_curA, s_nxtA = s_nxtA, s_curA
                s_curB, s_nxtB = s_nxtB, s_curB
                last_sqrt = i_sqB

            # trig passes after the sqrt chain (table-set order)
            sin_d = wp.tile([P, F], f32, tag="W1p", bufs=1)
            i_sind = sc.activation(sin_d[:], dc[:], AF.Sin)
            add_dep_helper(i_sind.ins, last_sqrt.ins,
                           reason="act set order sqrt->trig")
            cos_d = wp.tile([P, F], f32, tag="W2p", bufs=1)
            i_cosd = sc.activation(cos_d[:], dc[:], AF.Sin, bias=hpiv)
            rc = wp.tile([P, F], f32, tag="W3", bufs=1)
            v.reciprocal_approx_fast(rc[:], cos_d[:])
            tan045 = wp.tile([P, F], f32, tag="t045", bufs=1)
            v.scalar_tensor_tensor(tan045[:], sin_d[:], 0.45, rc[:],
                                   A.mult, A.mult)

            t453 = tan045[:].rearrange("p (n h) -> p n h", n=NPT)
            d3 = delta[:].rearrange("p (n h) -> p n h", n=NPT)
            fbb3 = fb[:].rearrange("p (n h) -> p n h", n=NPT)
            fxx3 = fx[:].rearrange("p (n h) -> p n h", n=NPT)

            first_arctan = [None]

            # ---------- phase D/E: per-chunk pipeline + staging + DMA ----------
            for g in range(G):
                n0 = g * NG
                us = u3[:, n0:n0 + NG, :].rearrange("p n h -> p (n h)")
                t45c = t453[:, n0:n0 + NG, :].rearrange("p n h -> p (n h)")

                # per-chunk yaw chain: s2 = relu(u) (exact; no ACT sqrt)
                s2c = cp.tile([P, CF], f32, tag="s2c", bufs=2)
                v.tensor_scalar(s2c[:], us, 0.0, None, A.max)
                mch = cp.tile([P, CF], f32, tag="mch", bufs=1)
                v.tensor_scalar(mch[:], s2c[:], 2.0, None, A.max)
                imc = cp.tile([P, CF], f32, tag="imc", bufs=1)
                v.reciprocal_approx_fast(imc[:], mch[:])
                rawc = cp.tile([P, CF], f32, tag="rawc", bufs=1)
                v.scalar_tensor_tensor(rawc[:], s2c[:], 1.0 / (0.45 * WB),
                                       t45c, A.mult, A.mult)
                clpc = cp.tile([P, CF], f32, tag="clpc", bufs=1)
                v.tensor_scalar(clpc[:], rawc[:], 1.0, -1.0, A.min, A.max)
                ylc = cp.tile([P, CF], f32, tag="ylc", bufs=1)
                v.tensor_scalar(ylc[:], imc[:], f981, None, A.mult)
                yawc = cp.tile([P, CF], f32, tag="yawc", bufs=2)
                v.scalar_tensor_tensor(yawc[:], ylc[:], 0.15, clpc[:],
                                       A.max, A.mult)
                ys = yawc[:]
                ss = s2c[:]

                stg = sp.tile([P, CHW], f32, tag="stg")
                s4 = stg[:].rearrange("p (n t c) -> p n t c", n=NG, t=HP1)

                # t=0 slice: 12 channels = x0 row, broadcast over rollouts
                x0b = x0s[:, None, 0:12].broadcast_to([P, NG, 12])
                v.tensor_scalar(s4[:, :, 0, :], x0b, 1.0, None, A.mult)

                # control-only channels (strided copies into staging)
                gp.tensor_scalar(s4[:, :, 1:, 9], d3[:, n0:n0 + NG, :],
                                 1.0, None, A.mult)
                gp.tensor_scalar(s4[:, :, 1:, 10], fbb3[:, n0:n0 + NG, :],
                                 1.0, None, A.mult)
                gp.tensor_scalar(s4[:, :, 1:, 11], fxx3[:, n0:n0 + NG, :],
                                 1.0, None, A.mult)
                i_bt = sc.activation(s4[:, :, 1:, 8], t453[:, n0:n0 + NG, :],
                                     AF.Arctan)
                if first_arctan[0] is None:
                    first_arctan[0] = i_bt
                    add_dep_helper(i_bt.ins, last_sqrt.ins,
                                   reason="act set order sqrt->trig")
                sc.copy(s4[:, :, 1:, 5],
                        yawc[:].rearrange("p (n h) -> p n h", n=NG))

                # psi = psi0 + 0.1 * segmented-cumsum(yawr)
                Fp = cp.tile([P, CF], f32, tag="Fp", bufs=2)
                v.tensor_tensor_scan(Fp[:], maskc[:], ys, 0.0, A.mult, A.add)
                sc.activation(s4[:, :, 1:, 2],
                              Fp[:].rearrange("p (n h) -> p n h", n=NG),
                              AF.Identity, bias=psi0, scale=DT)

                # arg = psi + beta ; wrap into [-pi, pi] for ACT sin
                argc = ps.tile([P, CF], f32, tag="pst")
                v.tensor_tensor(argc[:].rearrange("p (n h) -> p n h", n=NG),
                                s4[:, :, 1:, 2], s4[:, :, 1:, 8], A.add)
                argw = ps.tile([P, CF], f32, tag="pst")
                v.add_range_wrap(argw[:], argc[:], 0.0, PI, 2 * PI)
                cwv = ps.tile([P, CF], f32, tag="pst")
                v.add_range_wrap(cwv[:], argc[:], PI / 2, PI, 2 * PI)
                sinA = cp.tile([P, CF], f32, tag="sinA")
                sc.activation(sinA[:], argw[:], AF.Sin)
                cosA = cp.tile([P, CF], f32, tag="cosA")
                sc.activation(cosA[:], cwv[:], AF.Sin)

                vx2 = cp.tile([P, CF], f32, tag="vx2")
                v.tensor_tensor(vx2[:], ss, cosA[:], A.mult)
                vy2 = cp.tile([P, CF], f32, tag="vy2")
                v.tensor_tensor(vy2[:], ss, sinA[:], A.mult)

                vx3 = vx2[:].rearrange("p (n h) -> p n h", n=NG)
                vy3 = vy2[:].rearrange("p (n h) -> p n h", n=NG)
                sc.copy(s4[:, :, 1:, 3], vx3)
                sc.copy(s4[:, :, 1:, 4], vy3)

                # px/py via segmented cumsum of vx2/vy2
                Fx = cp.tile([P, CF], f32, tag="Fx", bufs=2)
                v.tensor_tensor_scan(Fx[:], maskc[:], vx2[:], 0.0, A.mult, A.add)
                sc.activation(s4[:, :, 1:, 0],
                              Fx[:].rearrange("p (n h) -> p n h", n=NG),
                              AF.Identity, bias=px0, scale=DT)
                Fy = cp.tile([P, CF], f32, tag="Fy", bufs=2)
                v.tensor_tensor_scan(Fy[:], maskc[:], vy2[:], 0.0, A.mult, A.add)
                sc.activation(s4[:, :, 1:, 1],
                              Fy[:].rearrange("p (n h) -> p n h", n=NG),
                              AF.Identity, bias=py0, scale=DT)

                # ax/ay: shifted diffs (t>=1); t=0 against vx0/vy0
                dxc = cp.tile([P, NG * (H - 1)], f32, tag="ddx", bufs=1)
                d3c = dxc[:].rearrange("p (n h) -> p n h", n=NG)
                gp.tensor_tensor(d3c, vx3[:, :, 1:], vx3[:, :, :H - 1], A.subtract)
                v.tensor_scalar(s4[:, :, 2:, 6], d3c, 10.0, None, A.mult)
                v.tensor_scalar(s4[:, :, 1, 6], vx3[:, :, 0], 10.0, vx010,
                                A.mult, A.subtract)
                dyc = cp.tile([P, NG * (H - 1)], f32, tag="ddy", bufs=1)
                dy3 = dyc[:].rearrange("p (n h) -> p n h", n=NG)
                gp.tensor_tensor(dy3, vy3[:, :, 1:], vy3[:, :, :H - 1], A.subtract)
                v.tensor_scalar(s4[:, :, 2:, 7], dy3, 10.0, None, A.mult)
                v.tensor_scalar(s4[:, :, 1, 7], vy3[:, :, 0], 10.0, vy010,
                                A.mult, A.subtract)

                sy.dma_start(out=out_d[:, g * CHW:(g + 1) * CHW], in_=stg[:])

    nc.compile()
    return nc


def _get_built():
    global _BUILT
    if _BUILT is None:
        _BUILT = _build_kernel()
    return _BUILT


def _run(x0, controls, deg, trace=False):
    from concourse.bass_utils import run_bass_kernel_spmd

    x0 = np.ascontiguousarray(x0, dtype=np.float32)
    controls = np.ascontiguousarray(controls, dtype=np.float32)
    deg = np.ascontiguousarray(deg, dtype=np.float32)

    nc = _get_built()
    in_maps = []
    for c in range(NCORES):
        sl = slice(c * BC, (c + 1) * BC)
        ctrl_c = controls[sl].reshape(R, H * 3).reshape(P, NPT * H * 3)
        x0p = np.repeat(x0[sl], P // BC, axis=0)      # [128, 12]
        degp = np.repeat(deg[sl], P // BC, axis=0)    # [128, 5]
        in_maps.append({
            "ctrl": np.ascontiguousarray(ctrl_c),
            "x0p": np.ascontiguousarray(x0p),
            "degp": np.ascontiguousarray(degp),
        })

    res = run_bass_kernel_spmd(nc, in_maps, list(range(NCORES)), trace=trace)
    outs = []
    for c in range(NCORES):
        o = np.asarray(res.results[c]["out"])
        outs.append(o.reshape(R, HP1, CW).reshape(BC, L, HP1, CW))
    return np.concatenate(outs, axis=0), res


def kernel(x0: np.ndarray, controls: np.ndarray, deg: np.ndarray) -> np.ndarray:
    out, _ = _run(x0, controls, deg)
    return out


if __name__ == "__main__":
    rng = np.random.default_rng(0)
    x0 = rng.standard_normal((B, 12)).astype(np.float32)
    controls = rng.standard_normal((B, L, H, 3)).astype(np.float32)
    deg = rng.random((B, 5)).astype(np.float32)
    out = kernel(x0, controls, deg)
    print("out", out.shape, out.dtype)



# revision 8
# speedup vs baseline: 1.8725x; 1.8725x over previous
"""Degraded bicycle rollout kernel for Trainium2 (8 NeuronCores, data-parallel on batch).

v2 restructure vs the serial-chain baseline:
  - The eps-free speed recurrence s' = max(s + a, 0) is computed by ONE
    hardware tensor_tensor_scan per chunk, via an 81-slot padded layout:
    slot 0 of each rollout holds -60000 (forces the running state to clamp
    to 0 = segment reset) and slot 1 is pre-biased with 10*s0. Dropping the
    reference's per-step sqrt(x^2+1e-6) costs ~1.5e-3 rel err (tol 2e-2)
    and removes the 80-step serial ACT-sqrt chain entirely.
  - The whole speed chain runs x10-scaled (a' = 10*acc*DT) so vxD = s2'*cosA
    is directly the ax/ay difference operand (ax = vxD_t - vxD_{t-1}) and
    px = 0.01*cumsum(vxD) + px0; no separate x10 passes.
  - Controls are DMA'd as fp16 (host-converted) halving input traffic;
    intermediates are fp16 where tolerance allows, which enables the DVE
    2x/4x perf modes for tensor_scalar/tensor_tensor ops.
  - Per-batch scalars (scales, s0, psi0, ...) are precomputed on host into
    one [128,12] tensor: partition p's rollouts all share batch p//2.
  - ACT work is split into two table phases (sigmoid_and_others for
    tanh/sigmoid, then trig_and_small for sin/arctan/identity) so exactly
    two LoadActFuncSet instructions are emitted.

Layout per core: 4096 rollouts = 128 partitions x 32; partition p holds
rollouts p*32..p*32+31, all from batch bc = p//2. Free dim rollout-major
f = n*80 + t. Output staged (n, t, c)-interleaved, DMA'd per chunk.
"""

import sys

sys.path.insert(0, "/opt/trn_rl_repo")

import numpy as np

B, L, H = 512, 64, 80
NCORES = 8
BC = B // NCORES          # 64 batches per core
R = BC * L                # 4096 rollouts per core
P = 128
NPT = R // P              # 32 rollouts per partition
F = NPT * H               # 2560 elements per partition
HP1 = H + 1
CW = 12                   # output channels
SW = HP1 * CW             # 972 staging words per rollout
G = 4                     # chunks
NG = NPT // G             # 8 rollouts per partition per chunk
CF = NG * H               # 640 free elems per chunk
CFP = NG * (H + 1)        # 648 padded elems per chunk (81-slot rollouts)
CHW = NG * SW             # 7776 staging words per partition per chunk
DT = 0.1
WB = 2.8
PI = float(np.pi)
PAD = -60000.0            # fp16-safe segment-reset value
SINSC = 0.999999          # keeps wrapped args strictly inside [-pi, pi]

_BUILT = None


def _build_kernel():
    import concourse.bass as bass
    import concourse.bacc as bacc
    import concourse.mybir as mybir
    from concourse.tile import TileContext
    from concourse.tile_rust import add_dep_helper

    f32 = mybir.dt.float32
    f16 = mybir.dt.float16
    A = mybir.AluOpType
    AF = mybir.ActivationFunctionType

    nc = bacc.Bacc(None, target_bir_lowering=False)
    ctrl_d = nc.declare_dram_parameter("ctrl", [P, NPT * H * 3], f16, isOutput=False)
    x0_d = nc.declare_dram_parameter("x0p", [P, 12], f32, isOutput=False)
    scal_d = nc.declare_dram_parameter("scal", [P, 12], f32, isOutput=False)
    out_d = nc.declare_dram_parameter("out", [P, NPT * SW], f32, isOutput=True)

    with TileContext(nc) as tc:
        v = nc.vector
        sc = nc.scalar
        gp = nc.gpsimd
        sy = nc.sync

        with tc.tile_pool(name="pers", bufs=1) as pp, \
             tc.tile_pool(name="ctrlp", bufs=2) as ctp, \
             tc.tile_pool(name="w1a", bufs=2) as w1p, \
             tc.tile_pool(name="w1b", bufs=4) as w1q, \
             tc.tile_pool(name="w2", bufs=2) as w2p, \
             tc.tile_pool(name="stgp", bufs=2) as sp:

            # ---------- persistent scalars / constants ----------
            x0s = pp.tile([P, 12], f32, tag="x0s")
            scal = pp.tile([P, 12], f32, tag="scal")
            sy.dma_start(out=x0s[:], in_=x0_d[:])
            sy.dma_start(out=scal[:], in_=scal_d[:])
            steer = scal[:, 0:1]
            brake = scal[:, 1:2]
            thr = scal[:, 2:3]
            lo75 = scal[:, 3:4]      # -7.5*friction  (x10 accDT lower bound)
            f981 = scal[:, 4:5]      # 9.81*friction
            s0x10 = scal[:, 5:6]     # 10*sqrt(vx0^2+vy0^2+1e-6)
            psi0 = scal[:, 6:7]
            px0 = scal[:, 7:8]
            py0 = scal[:, 8:9]
            vx010 = scal[:, 9:10]    # 10*vx0
            vy010 = scal[:, 10:11]   # 10*vy0
            psi010 = scal[:, 11:12]  # psi0/DT

            hpiv = pp.tile([P, 1], f32, tag="hpiv")
            v.memset(hpiv[:], PI / 2)

            maskc = pp.tile([P, CF], f16, tag="maskc")
            v.memset(maskc[:], 1.0)
            mc3 = maskc[:].rearrange("p (n h) -> p n h", n=NG)
            v.memset(mc3[:, :, 0:1], 0.0)

            zeros = pp.tile([P, CFP], f16, tag="zeros")
            v.memset(zeros[:], 0.0)

            apad = pp.tile([P, NPT * (H + 1)], f16, tag="apad")
            ap3 = apad[:].rearrange("p (n h) -> p n h", n=NPT)
            v.memset(ap3[:, :, 0:1], PAD)
            s2pad = pp.tile([P, NPT * (H + 1)], f16, tag="s2pad")
            s2p3 = s2pad[:].rearrange("p (n h) -> p n h", n=NPT)

            # ---------- input DMAs (per chunk) ----------
            CTW = NG * H * 3
            ctiles = []
            for g in range(G):
                ct = ctp.tile([P, CTW], f16, tag="ctrl")
                sy.dma_start(out=ct[:], in_=ctrl_d[:, g * CTW:(g + 1) * CTW])
                ctiles.append(ct)

            # ---------- wave 1 (table set: sigmoid_and_others) ----------
            last_w1_act = None
            dDs, fbDs, fxDs, dcs = [], [], [], []
            for g in range(G):
                c4 = ctiles[g][:].rearrange("p (n h c) -> p n h c", n=NG, h=H)
                th = w1p.tile([P, CF], f16, tag="th")
                th3 = th[:].rearrange("p (n h) -> p n h", n=NG)
                sc.activation(th3, c4[:, :, :, 0], AF.Tanh)
                sg1 = w1p.tile([P, CF], f16, tag="sg1")
                sg13 = sg1[:].rearrange("p (n h) -> p n h", n=NG)
                sc.activation(sg13, c4[:, :, :, 1], AF.Sigmoid)
                sg2 = w1p.tile([P, CF], f16, tag="sg2")
                sg23 = sg2[:].rearrange("p (n h) -> p n h", n=NG)
                i_sg2 = sc.activation(sg23, c4[:, :, :, 2], AF.Sigmoid)
                last_w1_act = i_sg2

                dD = w1q.tile([P, CF], f16, tag="dD")
                v.tensor_scalar(dD[:], th[:], steer, None, A.mult)
                dc = w1q.tile([P, CF], f16, tag="dc")
                v.tensor_scalar(dc[:], dD[:], 0.75, -0.75, A.min, A.max)
                fbD = w1q.tile([P, CF], f16, tag="fbD")
                v.tensor_scalar(fbD[:], sg1[:], brake, None, A.mult)
                fxD = w1q.tile([P, CF], f16, tag="fxD")
                v.tensor_scalar(fxD[:], sg2[:], thr, None, A.mult)
                fb65 = w1p.tile([P, CF], f16, tag="fb65")
                v.tensor_scalar(fb65[:], fbD[:], 6.5, None, A.mult)
                w1t = w1p.tile([P, CF], f16, tag="w1t")
                v.tensor_scalar(w1t[:], fxD[:], 2.8, None, A.mult)
                t3 = w1p.tile([P, CF], f16, tag="t3")
                v.tensor_tensor(t3[:], w1t[:], fb65[:], A.subtract)
                apg = ap3[:, g * NG:(g + 1) * NG, :]
                t33 = t3[:].rearrange("p (n h) -> p n h", n=NG)
                v.tensor_scalar(apg[:, :, 1:], t33, lo75, None, A.max)
                v.tensor_scalar(apg[:, :, 1:2], apg[:, :, 1:2], s0x10, None,
                                A.add)
                dDs.append(dD)
                fbDs.append(fbD)
                fxDs.append(fxD)
                dcs.append(dc)

            # ---------- wave 2 (table set: trig_and_small) ----------
            first_sin = [None]
            for g in range(G):
                n0 = g * NG
                stg = sp.tile([P, CHW], f32, tag="stg")
                s4 = stg[:].rearrange("p (n t c) -> p n t c", n=NG, t=HP1)

                # control-only channels + t=0 slice
                gp.tensor_scalar(s4[:, :, 1:, 9],
                                 dDs[g][:].rearrange("p (n h) -> p n h", n=NG),
                                 1.0, None, A.mult)
                gp.tensor_scalar(s4[:, :, 1:, 10],
                                 fbDs[g][:].rearrange("p (n h) -> p n h", n=NG),
                                 1.0, None, A.mult)
                gp.tensor_scalar(s4[:, :, 1:, 11],
                                 fxDs[g][:].rearrange("p (n h) -> p n h", n=NG),
                                 1.0, None, A.mult)
                x0b = x0s[:, None, 0:12].broadcast_to([P, NG, 12])
                gp.tensor_scalar(s4[:, :, 0, :], x0b, 1.0, None, A.mult)

                # tan/beta path
                sind = w2p.tile([P, CF], f32, tag="sind")
                i_sind = sc.activation(sind[:], dcs[g][:], AF.Sin)
                if first_sin[0] is None:
                    first_sin[0] = i_sind
                    add_dep_helper(i_sind.ins, last_w1_act.ins,
                                   reason="act table order sigmoid->trig")
                cosd = w2p.tile([P, CF], f32, tag="cosd")
                sc.activation(cosd[:], dcs[g][:], AF.Sin, bias=hpiv[:, 0:1])
                rc = w2p.tile([P, CF], f32, tag="rc")
                v.reciprocal_approx_fast(rc[:], cosd[:])
                t45c = w2p.tile([P, CF], f16, tag="t45c")
                v.scalar_tensor_tensor(t45c[:], sind[:], 0.45 / 12.6, rc[:],
                                       A.mult, A.mult)
                t45c3 = t45c[:].rearrange("p (n h) -> p n h", n=NG)
                sc.activation(s4[:, :, 1:, 8], t45c3, AF.Arctan, scale=12.6)

                # speed scan (x10-scaled, eps-free, padded-slot resets)
                v.tensor_tensor_scan(s2pad[:, g * CFP:(g + 1) * CFP],
                                     apad[:, g * CFP:(g + 1) * CFP],
                                     zeros[:], 0.0, A.add, A.max)
                s2v = s2p3[:, n0:n0 + NG, 1:]   # [P, NG, 80] fp16, x10

                # yaw chain
                mch = w2p.tile([P, CF], f32, tag="mch")
                gp.tensor_scalar(mch[:].rearrange("p (n h) -> p n h", n=NG),
                                 s2v, 0.1, 2.0, A.mult, A.max)
                imc = w2p.tile([P, CF], f32, tag="imc")
                v.reciprocal_approx_fast(imc[:], mch[:])
                ylc = w2p.tile([P, CF], f16, tag="ylc")
                gp.tensor_scalar(ylc[:], imc[:], f981, 0.15, A.mult, A.max)
                rawc = w2p.tile([P, CF], f16, tag="rawc")
                v.tensor_tensor(rawc[:].rearrange("p (n h) -> p n h", n=NG),
                                s2v, t45c3, A.mult)
                clpc = w2p.tile([P, CF], f16, tag="clpc")
                v.tensor_scalar(clpc[:], rawc[:], 1.0, -1.0, A.min, A.max)
                yawD = w2p.tile([P, CF], f16, tag="yawD")
                yawD3 = yawD[:].rearrange("p (n h) -> p n h", n=NG)
                v.tensor_tensor(yawD[:], clpc[:], ylc[:], A.mult)
                gp.tensor_scalar(s4[:, :, 1:, 5], yawD3, 1.0, None, A.mult)

                # psi / heading: psi0 folded into the cumsum via a first-slot
                # bias (after the unbiased yaw has been staged to ch5)
                v.tensor_scalar(yawD3[:, :, 0:1], yawD3[:, :, 0:1], psi010,
                                None, A.add)
                Pp = w2p.tile([P, CF], f32, tag="Pp")
                v.tensor_tensor_scan(Pp[:], maskc[:], yawD[:], 0.0,
                                     A.mult, A.add)
                Pp3 = Pp[:].rearrange("p (n h) -> p n h", n=NG)
                sc.activation(s4[:, :, 1:, 2], Pp3, AF.Identity, scale=DT)
                q = w2p.tile([P, CF], f32, tag="q")
                q3 = q[:].rearrange("p (n h) -> p n h", n=NG)
                v.tensor_tensor(q3, s4[:, :, 1:, 2], s4[:, :, 1:, 8], A.add)
                wrapS = w2p.tile([P, CF], f32, tag="wrapS")
                v.add_range_wrap(wrapS[:], q[:], 0.0, PI, 2 * PI)
                wrapC = w2p.tile([P, CF], f32, tag="wrapC")
                v.add_range_wrap(wrapC[:], q[:], PI / 2, PI, 2 * PI)
                sinA = w2p.tile([P, CF], f16, tag="sinA")
                sc.activation(sinA[:], wrapS[:], AF.Sin, scale=SINSC)
                cosA = w2p.tile([P, CF], f16, tag="cosA")
                sc.activation(cosA[:], wrapC[:], AF.Sin, scale=SINSC)

                # velocities (x10) and outputs
                vxD = w2p.tile([P, CF], f16, tag="vxD")
                vxD3 = vxD[:].rearrange("p (n h) -> p n h", n=NG)
                v.tensor_tensor(vxD3, s2v, cosA[:].rearrange(
                    "p (n h) -> p n h", n=NG), A.mult)
                vyD = w2p.tile([P, CF], f16, tag="vyD")
                vyD3 = vyD[:].rearrange("p (n h) -> p n h", n=NG)
                gp.tensor_tensor(vyD3, s2v, sinA[:].rearrange(
                    "p (n h) -> p n h", n=NG), A.mult)
                sc.activation(s4[:, :, 1:, 3], vxD3, AF.Copy, scale=0.1)
                sc.activation(s4[:, :, 1:, 4], vyD3, AF.Copy, scale=0.1)

                Fx = w2p.tile([P, CF], f32, tag="Fx")
                v.tensor_tensor_scan(Fx[:], maskc[:], vxD[:], 0.0,
                                     A.mult, A.add)
                sc.activation(s4[:, :, 1:, 0],
                              Fx[:].rearrange("p (n h) -> p n h", n=NG),
                              AF.Identity, bias=px0, scale=DT * 0.1)
                Fy = w2p.tile([P, CF], f32, tag="Fy")
                v.tensor_tensor_scan(Fy[:], maskc[:], vyD[:], 0.0,
                                     A.mult, A.add)
                sc.activation(s4[:, :, 1:, 1],
                              Fy[:].rearrange("p (n h) -> p n h", n=NG),
                              AF.Identity, bias=py0, scale=DT * 0.1)

                # accelerations: x10 diffs of the x10 velocities
                v.tensor_tensor(s4[:, :, 2:, 6], vxD3[:, :, 1:],
                                vxD3[:, :, :H - 1], A.subtract)
                v.tensor_scalar(s4[:, :, 1, 6], vxD3[:, :, 0], 1.0, vx010,
                                A.mult, A.subtract)
                gp.tensor_tensor(s4[:, :, 2:, 7], vyD3[:, :, 1:],
                                 vyD3[:, :, :H - 1], A.subtract)
                gp.tensor_scalar(s4[:, :, 1, 7], vyD3[:, :, 0], 1.0, vy010,
                                 A.mult, A.subtract)

                sy.dma_start(out=out_d[:, g * CHW:(g + 1) * CHW], in_=stg[:])

    nc.compile()
    return nc


def _get_built():
    global _BUILT
    if _BUILT is None:
        _BUILT = _build_kernel()
    return _BUILT


def _run(x0, controls, deg, trace=False):
    from concourse.bass_utils import run_bass_kernel_spmd

    x0 = np.ascontiguousarray(x0, dtype=np.float32)
    controls = np.ascontiguousarray(controls, dtype=np.float32)
    deg = np.ascontiguousarray(deg, dtype=np.float32)

    nc = _get_built()
    in_maps = []
    for c in range(NCORES):
        sl = slice(c * BC, (c + 1) * BC)
        ctrl_c = controls[sl].reshape(R, H * 3).reshape(P, NPT * H * 3)
        x0p = np.repeat(x0[sl], P // BC, axis=0)      # [128, 12]
        degp = np.repeat(deg[sl], P // BC, axis=0)    # [128, 5]
        x0f = x0p.astype(np.float64)
        scal = np.zeros((P, 12), dtype=np.float32)
        fric = np.maximum(degp[:, 4], 0.1)
        scal[:, 0] = np.maximum(degp[:, 0], 0.05)            # steer
        scal[:, 1] = np.maximum(degp[:, 1], 0.05)            # brake
        scal[:, 2] = np.maximum(degp[:, 2], 0.05)            # thr
        scal[:, 3] = -7.5 * fric                             # lo (x10)
        scal[:, 4] = 9.81 * fric                             # f981
        scal[:, 5] = 10.0 * np.sqrt(x0f[:, 3] ** 2 + x0f[:, 4] ** 2 + 1e-6)
        scal[:, 6] = x0p[:, 2]                               # psi0
        scal[:, 7] = x0p[:, 0]                               # px0
        scal[:, 8] = x0p[:, 1]                               # py0
        scal[:, 9] = 10.0 * x0p[:, 3]                        # 10*vx0
        scal[:, 10] = 10.0 * x0p[:, 4]                       # 10*vy0
        scal[:, 11] = x0p[:, 2] / DT                         # psi0/DT
        in_maps.append({
            "ctrl": np.ascontiguousarray(ctrl_c.astype(np.float16)),
            "x0p": np.ascontiguousarray(x0p),
            "scal": scal,
        })

    res = run_bass_kernel_spmd(nc, in_maps, list(range(NCORES)), trace=trace)
    outs = []
    for c in range(NCORES):
        o = np.asarray(res.results[c]["out"])
        outs.append(o.reshape(R, HP1, CW).reshape(BC, L, HP1, CW))
    return np.concatenate(outs, axis=0), res


def kernel(x0: np.ndarray, controls: np.ndarray, deg: np.ndarray) -> np.ndarray:
    out, _ = _run(x0, controls, deg)
    return out


if __name__ == "__main__":
    rng = np.random.default_rng(0)
    x0 = rng.standard_normal((B, 12)).astype(np.float32)
    controls = rng.standard_normal((B, L, H, 3)).astype(np.float32)
    deg = rng.random((B, 5)).astype(np.float32)
    out = kernel(x0, controls, deg)
    print("out", out.shape, out.dtype)


# revision 16
# speedup vs baseline: 1.9696x; 1.0519x over previous
"""Degraded bicycle rollout kernel for Trainium2 (8 NeuronCores, data-parallel on batch).

v2 restructure vs the serial-chain baseline:
  - The eps-free speed recurrence s' = max(s + a, 0) is computed by ONE
    hardware tensor_tensor_scan per chunk, via an 81-slot padded layout:
    slot 0 of each rollout holds -60000 (forces the running state to clamp
    to 0 = segment reset) and slot 1 is pre-biased with 10*s0. Dropping the
    reference's per-step sqrt(x^2+1e-6) costs ~1.5e-3 rel err (tol 2e-2)
    and removes the 80-step serial ACT-sqrt chain entirely.
  - The whole speed chain runs x10-scaled (a' = 10*acc*DT) so vxD = s2'*cosA
    is directly the ax/ay difference operand (ax = vxD_t - vxD_{t-1}) and
    px = 0.01*cumsum(vxD) + px0; no separate x10 passes.
  - Controls are DMA'd as fp16 (host-converted) halving input traffic;
    intermediates are fp16 where tolerance allows, which enables the DVE
    2x/4x perf modes for tensor_scalar/tensor_tensor ops.
  - Per-batch scalars (scales, s0, psi0, ...) are precomputed on host into
    one [128,12] tensor: partition p's rollouts all share batch p//2.
  - ACT work is split into two table phases (sigmoid_and_others for
    tanh/sigmoid, then trig_and_small for sin/arctan/identity) so exactly
    two LoadActFuncSet instructions are emitted.

Layout per core: 4096 rollouts = 128 partitions x 32; partition p holds
rollouts p*32..p*32+31, all from batch bc = p//2. Free dim rollout-major
f = n*80 + t. Output staged (n, t, c)-interleaved, DMA'd per chunk.
"""

import sys

sys.path.insert(0, "/opt/trn_rl_repo")

import numpy as np

B, L, H = 512, 64, 80
NCORES = 8
BC = B // NCORES          # 64 batches per core
R = BC * L                # 4096 rollouts per core
P = 128
NPT = R // P              # 32 rollouts per partition
F = NPT * H               # 2560 elements per partition
HP1 = H + 1
CW = 12                   # output channels
SW = HP1 * CW             # 972 staging words per rollout
G = 4                     # chunks
NG = NPT // G             # 8 rollouts per partition per chunk
CF = NG * H               # 640 free elems per chunk
CFP = NG * (H + 1)        # 648 padded elems per chunk (81-slot rollouts)
CHW = NG * SW             # 7776 staging words per partition per chunk
DT = 0.1
WB = 2.8
PI = float(np.pi)
PAD = -60000.0            # fp16-safe segment-reset value
SINSC = 0.999999          # keeps wrapped args strictly inside [-pi, pi]

_BUILT = None


def _build_kernel():
    import concourse.bass as bass
    import concourse.bacc as bacc
    import concourse.mybir as mybir
    from concourse.tile import TileContext
    from concourse.tile_rust import add_dep_helper

    f32 = mybir.dt.float32
    f16 = mybir.dt.float16
    A = mybir.AluOpType
    AF = mybir.ActivationFunctionType

    nc = bacc.Bacc(None, target_bir_lowering=False)
    ctrl_d = nc.declare_dram_parameter("ctrl", [P, NPT * H * 3], f16, isOutput=False)
    x0_d = nc.declare_dram_parameter("x0p", [P, 12], f32, isOutput=False)
    scal_d = nc.declare_dram_parameter("scal", [P, 16], f32, isOutput=False)
    out_d = nc.declare_dram_parameter("out", [P, NPT * SW], f32, isOutput=True)

    with TileContext(nc) as tc:
        v = nc.vector
        sc = nc.scalar
        gp = nc.gpsimd
        sy = nc.sync

        with tc.tile_pool(name="pers", bufs=1) as pp, \
             tc.tile_pool(name="ctrlp", bufs=4) as ctp, \
             tc.tile_pool(name="w1a", bufs=2) as w1p, \
             tc.tile_pool(name="w1b", bufs=4) as w1q, \
             tc.tile_pool(name="w2", bufs=2) as w2p, \
             tc.tile_pool(name="stgp", bufs=2) as sp:

            # ---------- persistent scalars / constants ----------
            x0s = pp.tile([P, 12], f32, tag="x0s")
            scal = pp.tile([P, 16], f32, tag="scal")
            sy.dma_start(out=x0s[:], in_=x0_d[:])
            sy.dma_start(out=scal[:], in_=scal_d[:])
            steer = scal[:, 0:1]
            brake = scal[:, 1:2]
            thr = scal[:, 2:3]
            lo75 = scal[:, 3:4]      # -7.5*friction  (x10 accDT lower bound)
            f981 = scal[:, 4:5]      # 9.81*friction
            s0x10 = scal[:, 5:6]     # 10*sqrt(vx0^2+vy0^2+1e-6)
            psi0 = scal[:, 6:7]
            px0 = scal[:, 7:8]
            py0 = scal[:, 8:9]
            vx010 = scal[:, 9:10]    # 10*vx0
            vy010 = scal[:, 10:11]   # 10*vy0
            psi010 = scal[:, 11:12]  # psi0/DT
            hbrake = scal[:, 12:13]  # 0.5*brake
            hthr = scal[:, 13:14]    # 0.5*thr

            hpiv = pp.tile([P, 1], f32, tag="hpiv")
            v.memset(hpiv[:], PI / 2)

            maskc = pp.tile([P, CF], f16, tag="maskc")
            v.memset(maskc[:], 1.0)
            mc3 = maskc[:].rearrange("p (n h) -> p n h", n=NG)
            v.memset(mc3[:, :, 0:1], 0.0)

            zeros = pp.tile([P, CFP], f16, tag="zeros")
            v.memset(zeros[:], 0.0)

            apad = pp.tile([P, NPT * (H + 1)], f16, tag="apad")
            ap3 = apad[:].rearrange("p (n h) -> p n h", n=NPT)
            v.memset(ap3[:, :, 0:1], PAD)
            s2pad = pp.tile([P, NPT * (H + 1)], f16, tag="s2pad")
            s2p3 = s2pad[:].rearrange("p (n h) -> p n h", n=NPT)

            # ---------- input DMAs (per chunk) ----------
            CTW = NG * H * 3
            ctiles = []
            for g in range(G):
                ct = ctp.tile([P, CTW], f16, tag="ctrl")
                sy.dma_start(out=ct[:], in_=ctrl_d[:, g * CTW:(g + 1) * CTW])
                ctiles.append(ct)

            # ---------- wave 1 (all Tanh: table set silu_and_others, which
            # also holds Sin -> the sin chain needs no table switch; sigmoid
            # is computed as 0.5 + 0.5*tanh(u/2), folded into the TSP) ----
            last_w1_act = None
            dDs, fbDs, fxDs, dcs = [], [], [], []
            for g in range(G):
                c4 = ctiles[g][:].rearrange("p (n h c) -> p n h c", n=NG, h=H)
                th = w1p.tile([P, CF], f16, tag="th")
                th3 = th[:].rearrange("p (n h) -> p n h", n=NG)
                sc.activation(th3, c4[:, :, :, 0], AF.Tanh)
                sg1 = w1p.tile([P, CF], f16, tag="sg1")
                sg13 = sg1[:].rearrange("p (n h) -> p n h", n=NG)
                sc.activation(sg13, c4[:, :, :, 1], AF.Tanh, scale=0.5)
                sg2 = w1p.tile([P, CF], f16, tag="sg2")
                sg23 = sg2[:].rearrange("p (n h) -> p n h", n=NG)
                i_sg2 = sc.activation(sg23, c4[:, :, :, 2], AF.Tanh, scale=0.5)
                last_w1_act = i_sg2

                dD = w1q.tile([P, CF], f16, tag="dD")
                v.tensor_scalar(dD[:], th[:], steer, None, A.mult)
                dc = w1q.tile([P, CF], f16, tag="dc")
                v.tensor_scalar(dc[:], dD[:], 0.75, -0.75, A.min, A.max)
                fbD = w1q.tile([P, CF], f16, tag="fbD")
                v.tensor_scalar(fbD[:], sg1[:], hbrake, hbrake, A.mult, A.add)
                fxD = w1q.tile([P, CF], f16, tag="fxD")
                v.tensor_scalar(fxD[:], sg2[:], hthr, hthr, A.mult, A.add)
                fb65 = w1p.tile([P, CF], f16, tag="fb65")
                v.tensor_scalar(fb65[:], fbD[:], 6.5, None, A.mult)
                w1t = w1p.tile([P, CF], f16, tag="w1t")
                v.tensor_scalar(w1t[:], fxD[:], 2.8, None, A.mult)
                t3 = w1p.tile([P, CF], f16, tag="t3")
                v.tensor_tensor(t3[:], w1t[:], fb65[:], A.subtract)
                apg = ap3[:, g * NG:(g + 1) * NG, :]
                t33 = t3[:].rearrange("p (n h) -> p n h", n=NG)
                v.tensor_scalar(apg[:, :, 1:], t33, lo75, None, A.max)
                v.tensor_scalar(apg[:, :, 1:2], apg[:, :, 1:2], s0x10, None,
                                A.add)
                dDs.append(dD)
                fbDs.append(fbD)
                fxDs.append(fxD)
                dcs.append(dc)

            # ---------- wave 2 (arctan switches to trig_and_small; all other
            # ACT funcs here are in both sets, so exactly one switch) -------
            for g in range(G):
                n0 = g * NG
                stg = sp.tile([P, CHW], f32, tag="stg")
                s4 = stg[:].rearrange("p (n t c) -> p n t c", n=NG, t=HP1)

                # control-only channels + t=0 slice
                gp.tensor_scalar(s4[:, :, 1:, 9],
                                 dDs[g][:].rearrange("p (n h) -> p n h", n=NG),
                                 1.0, None, A.mult)
                gp.tensor_scalar(s4[:, :, 1:, 10],
                                 fbDs[g][:].rearrange("p (n h) -> p n h", n=NG),
                                 1.0, None, A.mult)
                gp.tensor_scalar(s4[:, :, 1:, 11],
                                 fxDs[g][:].rearrange("p (n h) -> p n h", n=NG),
                                 1.0, None, A.mult)
                x0b = x0s[:, None, 0:12].broadcast_to([P, NG, 12])
                gp.tensor_scalar(s4[:, :, 0, :], x0b, 1.0, None, A.mult)

                # speed scan (x10-scaled, eps-free, padded-slot resets) comes
                # first in the DVE stream: it depends only on wave 1
                v.tensor_tensor_scan(s2pad[:, g * CFP:(g + 1) * CFP],
                                     apad[:, g * CFP:(g + 1) * CFP],
                                     zeros[:], 0.0, A.add, A.max)
                s2v = s2p3[:, n0:n0 + NG, 1:]   # [P, NG, 80] fp16, x10

                # tan/beta path
                sind = w2p.tile([P, CF], f32, tag="sind")
                sc.activation(sind[:], dcs[g][:], AF.Sin)
                cosd = w2p.tile([P, CF], f32, tag="cosd")
                sc.activation(cosd[:], dcs[g][:], AF.Sin, bias=hpiv[:, 0:1])
                rc = w2p.tile([P, CF], f32, tag="rc")
                v.reciprocal_approx_fast(rc[:], cosd[:])
                t45c = w2p.tile([P, CF], f16, tag="t45c")
                v.scalar_tensor_tensor(t45c[:], sind[:], 0.45 / 12.6, rc[:],
                                       A.mult, A.mult)
                t45c3 = t45c[:].rearrange("p (n h) -> p n h", n=NG)
                i_bt = sc.activation(s4[:, :, 1:, 8], t45c3, AF.Arctan,
                                     scale=12.6)
                add_dep_helper(i_bt.ins, last_w1_act.ins,
                               reason="act table order tanh->trig")

                # yaw chain
                mch = w2p.tile([P, CF], f32, tag="mch")
                gp.tensor_scalar(mch[:].rearrange("p (n h) -> p n h", n=NG),
                                 s2v, 0.1, 2.0, A.mult, A.max)
                imc = w2p.tile([P, CF], f32, tag="imc")
                v.reciprocal_approx_fast(imc[:], mch[:])
                ylc = w2p.tile([P, CF], f16, tag="ylc")
                gp.tensor_scalar(ylc[:], imc[:], f981, 0.15, A.mult, A.max)
                rawc = w2p.tile([P, CF], f16, tag="rawc")
                v.tensor_tensor(rawc[:].rearrange("p (n h) -> p n h", n=NG),
                                s2v, t45c3, A.mult)
                clpc = w2p.tile([P, CF], f16, tag="clpc")
                v.tensor_scalar(clpc[:], rawc[:], 1.0, -1.0, A.min, A.max)
                yawD = w2p.tile([P, CF], f16, tag="yawD")
                yawD3 = yawD[:].rearrange("p (n h) -> p n h", n=NG)
                v.tensor_tensor(yawD[:], clpc[:], ylc[:], A.mult)
                gp.tensor_scalar(s4[:, :, 1:, 5], yawD3, 1.0, None, A.mult)

                # psi / heading: psi0 folded into the cumsum via a first-slot
                # bias (after the unbiased yaw has been staged to ch5)
                v.tensor_scalar(yawD3[:, :, 0:1], yawD3[:, :, 0:1], psi010,
                                None, A.add)
                Pp = w2p.tile([P, CF], f32, tag="Pp")
                v.tensor_tensor_scan(Pp[:], maskc[:], yawD[:], 0.0,
                                     A.mult, A.add)
                Pp3 = Pp[:].rearrange("p (n h) -> p n h", n=NG)
                sc.activation(s4[:, :, 1:, 2], Pp3, AF.Identity, scale=DT)
                q = w2p.tile([P, CF], f32, tag="q")
                q3 = q[:].rearrange("p (n h) -> p n h", n=NG)
                v.tensor_tensor(q3, s4[:, :, 1:, 2], s4[:, :, 1:, 8], A.add)
                wrapS = w2p.tile([P, CF], f32, tag="wrapS")
                v.add_range_wrap(wrapS[:], q[:], 0.0, PI, 2 * PI)
                wrapC = w2p.tile([P, CF], f32, tag="wrapC")
                v.add_range_wrap(wrapC[:], q[:], PI / 2, PI, 2 * PI)
                sinA = w2p.tile([P, CF], f16, tag="sinA")
                sc.activation(sinA[:], wrapS[:], AF.Sin, scale=SINSC)
                cosA = w2p.tile([P, CF], f16, tag="cosA")
                sc.activation(cosA[:], wrapC[:], AF.Sin, scale=SINSC)

                # velocities (x10) and outputs
                vxD = w2p.tile([P, CF], f16, tag="vxD")
                vxD3 = vxD[:].rearrange("p (n h) -> p n h", n=NG)
                v.tensor_tensor(vxD3, s2v, cosA[:].rearrange(
                    "p (n h) -> p n h", n=NG), A.mult)
                vyD = w2p.tile([P, CF], f16, tag="vyD")
                vyD3 = vyD[:].rearrange("p (n h) -> p n h", n=NG)
                gp.tensor_tensor(vyD3, s2v, sinA[:].rearrange(
                    "p (n h) -> p n h", n=NG), A.mult)
                sc.activation(s4[:, :, 1:, 3], vxD3, AF.Copy, scale=0.1)
                sc.activation(s4[:, :, 1:, 4], vyD3, AF.Copy, scale=0.1)

                Fx = w2p.tile([P, CF], f32, tag="Fx")
                v.tensor_tensor_scan(Fx[:], maskc[:], vxD[:], 0.0,
                                     A.mult, A.add)
                sc.activation(s4[:, :, 1:, 0],
                              Fx[:].rearrange("p (n h) -> p n h", n=NG),
                              AF.Identity, bias=px0, scale=DT * 0.1)
                Fy = w2p.tile([P, CF], f32, tag="Fy")
                v.tensor_tensor_scan(Fy[:], maskc[:], vyD[:], 0.0,
                                     A.mult, A.add)
                sc.activation(s4[:, :, 1:, 1],
                              Fy[:].rearrange("p (n h) -> p n h", n=NG),
                              AF.Identity, bias=py0, scale=DT * 0.1)

                # accelerations: x10 diffs of the x10 velocities
                v.tensor_tensor(s4[:, :, 2:, 6], vxD3[:, :, 1:],
                                vxD3[:, :, :H - 1], A.subtract)
                v.tensor_scalar(s4[:, :, 1, 6], vxD3[:, :, 0], 1.0, vx010,
                                A.mult, A.subtract)
                gp.tensor_tensor(s4[:, :, 2:, 7], vyD3[:, :, 1:],
                                 vyD3[:, :, :H - 1], A.subtract)
                gp.tensor_scalar(s4[:, :, 1, 7], vyD3[:, :, 0], 1.0, vy010,
                                 A.mult, A.subtract)

                sy.dma_start(out=out_d[:, g * CHW:(g + 1) * CHW], in_=stg[:])

    nc.compile()
    return nc


def _get_built():
    global _BUILT
    if _BUILT is None:
        _BUILT = _build_kernel()
    return _BUILT


def _run(x0, controls, deg, trace=False):
    from concourse.bass_utils import run_bass_kernel_spmd

    x0 = np.ascontiguousarray(x0, dtype=np.float32)
    controls = np.ascontiguousarray(controls, dtype=np.float32)
    deg = np.ascontiguousarray(deg, dtype=np.float32)

    nc = _get_built()
    in_maps = []
    for c in range(NCORES):
        sl = slice(c * BC, (c + 1) * BC)
        ctrl_c = controls[sl].reshape(R, H * 3).reshape(P, NPT * H * 3)
        x0p = np.repeat(x0[sl], P // BC, axis=0)      # [128, 12]
        degp = np.repeat(deg[sl], P // BC, axis=0)    # [128, 5]
        x0f = x0p.astype(np.float64)
        scal = np.zeros((P, 16), dtype=np.float32)
        fric = np.maximum(degp[:, 4], 0.1)
        scal[:, 0] = np.maximum(degp[:, 0], 0.05)            # steer
        scal[:, 1] = np.maximum(degp[:, 1], 0.05)            # brake
        scal[:, 2] = np.maximum(degp[:, 2], 0.05)            # thr
        scal[:, 3] = -7.5 * fric                             # lo (x10)
        scal[:, 4] = 9.81 * fric                             # f981
        scal[:, 5] = 10.0 * np.sqrt(x0f[:, 3] ** 2 + x0f[:, 4] ** 2 + 1e-6)
        scal[:, 6] = x0p[:, 2]                               # psi0
        scal[:, 7] = x0p[:, 0]                               # px0
        scal[:, 8] = x0p[:, 1]                               # py0
        scal[:, 9] = 10.0 * x0p[:, 3]                        # 10*vx0
        scal[:, 10] = 10.0 * x0p[:, 4]                       # 10*vy0
        scal[:, 11] = x0p[:, 2] / DT                         # psi0/DT
        scal[:, 12] = 0.5 * np.maximum(degp[:, 1], 0.05)     # brake/2
        scal[:, 13] = 0.5 * np.maximum(degp[:, 2], 0.05)     # thr/2
        in_maps.append({
            "ctrl": np.ascontiguousarray(ctrl_c.astype(np.float16)),
            "x0p": np.ascontiguousarray(x0p),
            "scal": scal,
        })

    res = run_bass_kernel_spmd(nc, in_maps, list(range(NCORES)), trace=trace)
    outs = []
    for c in range(NCORES):
        o = np.asarray(res.results[c]["out"])
        outs.append(o.reshape(R, HP1, CW).reshape(BC, L, HP1, CW))
    return np.concatenate(outs, axis=0), res


def kernel(x0: np.ndarray, controls: np.ndarray, deg: np.ndarray) -> np.ndarray:
    out, _ = _run(x0, controls, deg)
    return out


if __name__ == "__main__":
    rng = np.random.default_rng(0)
    x0 = rng.standard_normal((B, 12)).astype(np.float32)
    controls = rng.standard_normal((B, L, H, 3)).astype(np.float32)
    deg = rng.random((B, 5)).astype(np.float32)
    out = kernel(x0, controls, deg)
    print("out", out.shape, out.dtype)
